# revision 1
# baseline (speedup 1.0000x reference)
"""Trainium2 Bass kernel for nn_LstmRNN: 8-core tensor-parallel LSTM.

Strategy (8 NeuronCores, SPMD):
  - Tensor-parallel split of the 4H gate dimension: core j owns hidden-state
    slice j (128 of 1024 dims) and the matching 512 columns of W_ih / W_hh
    (gate order permuted host-side to [i, f, o, g] so the sigmoid covers
    contiguous blocks of the transposed gates).
  - Phase 1: x_gates^T = W_ih_local^T @ xs^T for all (t, b), bf16 matmuls,
    fp32 PSUM accumulate, bias fused into the PSUM->SBUF eviction.
  - Phase 2: 128 serial steps. Per step: inject x_gates into PSUM via an
    identity matmul (emitted first; overlaps the exchange window), accumulate
    8 K-chunk matmuls of h^T @ W_hh per gate into one PSUM bank per gate,
    sigmoid (ACT) + relu/mul/add (DVE) in fp32 overlapping the later gates'
    matmul groups, produce the core's h^T chunk [128, 64] in bf16, AllGather
    it across the 8 cores (bounce DRAM -> ncfw collective -> DRAM -> SBUF).
  - Phase 3: out = h @ W_o + b_o, bias via a K=1 ones-row matmul.

All matmul operands are bf16 (1 cycle/row on PE vs 4 for fp32); state c and
all elementwise math stay fp32. Validated vs the fp32 reference: max
absmax-relative error ~2.3e-3 (gate is 2e-2).

Measured on trn2 (neuron-profile): ~2.10 ms total; per step ~15.4 us =
~2.5 us matmul+EW (PE warm, 53 ns/MM issue) + ~12.9 us exchange (the
per-step 8-core AllGather path: ~0.6+1.3 us bounce-out, ~7.2 us ncfw
collective, ~0.6+2.3 us bounce-in incl. HBM completion receipt).

An alternative exchange="rdma" path (SBUF->SBUF remote_dma_broadcast,
per-step latency ~2-3 us in theory) is implemented and validated in
MultiCoreSim; on hardware the 8x singleton-destination variant was stable
but serialized deliveries (~51 us/step), and the single 8-destination
variant faulted the device (NRT_EXEC_UNIT_UNRECOVERABLE), so the collective
exchange remains the default.
"""

import sys

for _p in ("/opt/trn_rl_repo",):
    if _p not in sys.path:
        sys.path.insert(0, _p)

import numpy as np
import ml_dtypes

import concourse.bass as bass
import concourse.mybir as mybir
import concourse.tile as tile
from concourse import bacc
from concourse import bass_utils
from concourse.bass import _add_dep_helper

BF16 = ml_dtypes.bfloat16

B, T, I, H, O = 64, 128, 512, 1024, 512
NCORES = 8
HSL = H // NCORES          # 128 hidden dims per core
GCOLS = 4 * HSL            # 512 gate columns per core (i,f,o,g x 128)
NB = T * B // 512          # phase-1 512-wide col-chunks (16)

F32 = mybir.dt.float32
BF = mybir.dt.bfloat16
AF = mybir.ActivationFunctionType
ALU = mybir.AluOpType


def build_program(t_steps: int = T, exchange: str = "rdma"):
    """exchange: 'rdma' = SBUF->SBUF remote_dma_broadcast; 'cc' = ncfw AllGather."""
    nc = bacc.Bacc(
        "TRN2",
        target_bir_lowering=False,
        debug=False,
        num_devices=NCORES,
    )

    xs_t = nc.dram_tensor("xs_t", [I, T * B], BF, kind="ExternalInput")
    wih = nc.dram_tensor("wih", [I, GCOLS], BF, kind="ExternalInput")
    whh = nc.dram_tensor("whh", [H, GCOLS], BF, kind="ExternalInput")
    bias = nc.dram_tensor("bias", [HSL, 4], F32, kind="ExternalInput")
    wo = nc.dram_tensor("wo", [H, O], BF, kind="ExternalInput")
    bo = nc.dram_tensor("bo", [1, O], BF, kind="ExternalInput")
    ident = nc.dram_tensor("ident", [128, 128], BF, kind="ExternalInput")
    ones = nc.dram_tensor("ones", [1, B], BF, kind="ExternalInput")
    out = nc.dram_tensor("out", [B, O], F32, kind="ExternalOutput")

    with tile.TileContext(nc) as tc:
        with (
            tc.tile_pool(name="consts", bufs=1) as consts,
            tc.tile_pool(name="xg", bufs=1) as xgp,
            tc.tile_pool(name="xsp", bufs=2) as xsp,
            tc.tile_pool(name="psum", bufs=2, space="PSUM") as psp,
            tc.tile_pool(name="ew", bufs=2) as ew,
            tc.tile_pool(name="hall", bufs=2) as hallp,
            tc.tile_pool(name="dram", bufs=2, space="DRAM") as dram,
        ):
            # ---- constants into SBUF ----
            wih_sb = consts.tile([128, 4, GCOLS], BF)
            nc.sync.dma_start(wih_sb[:], wih.rearrange("(k p) c -> p k c", p=128))
            whh_sb = consts.tile([128, 8, GCOLS], BF)
            nc.sync.dma_start(whh_sb[:], whh.rearrange("(k p) c -> p k c", p=128))
            bias_sb = consts.tile([HSL, 4], F32)
            nc.sync.dma_start(bias_sb[:], bias[:, :])
            id_sb = consts.tile([128, 128], BF)
            nc.sync.dma_start(id_sb[:], ident[:, :])
            ones_sb = consts.tile([1, B], BF)
            nc.sync.dma_start(ones_sb[:], ones[:, :])
            wo_sb = consts.tile([128, 8, O], BF)
            nc.sync.dma_start(wo_sb[:], wo.rearrange("(k p) c -> p k c", p=128))
            bo_sb = consts.tile([1, O], BF)
            nc.sync.dma_start(bo_sb[:], bo[:, :])

            # x_gates^T per gate chunk: [128 gate-dims, t*64+b]
            xg = [
                xgp.tile([128, T * B], BF, tag=f"xg{g}", name=f"xg{g}")
                for g in range(4)
            ]

            # ---- phase 1: x_gates^T = W_ih_local^T @ xs^T (+ bias) ----
            # PSUM budget: 4 tags x bufs=2 = 8 banks, shared with phase 2/3.
            ptags = ["pA", "pB", "pC", "pD"]
            xs_r = xs_t.rearrange("(k p) n -> p k n", p=128)
            for blk in range(T * B // 1024):  # 8 panels of 1024 cols
                panel = xsp.tile([128, 4, 1024], BF, tag="panel")
                nc.sync.dma_start(
                    panel[:], xs_r[:, :, blk * 1024 : (blk + 1) * 1024]
                )
                for g in range(4):
                    for sub in range(2):
                        ps = psp.tile(
                            [128, 512], F32, tag=ptags[g], name=f"ps1_{g}"
                        )
                        for k in range(4):
                            nc.tensor.matmul(
                                ps[:],
                                wih_sb[:, k, g * 128 : (g + 1) * 128],
                                panel[:, k, sub * 512 : (sub + 1) * 512],
                                start=(k == 0),
                                stop=(k == 3),
                            )
                        col0 = blk * 1024 + sub * 512
                        nc.vector.tensor_scalar(
                            xg[g][:, col0 : col0 + 512],
                            ps[:],
                            bias_sb[:, g : g + 1],
                            None,
                            ALU.add,
                        )

            # ---- phase 2: recurrence ----
            h_all = hallp.tile([128, 8 * B], BF, tag="hall")
            nc.vector.memset(h_all[:], 0.0)
            c_prev = ew.tile([128, B], F32, tag="c")
            nc.vector.memset(c_prev[:], 0.0)

            if exchange == "rdma":
                # remote-DMA all-gather: per step, ONE 8-destination broadcast
                # (relative peers own^k on engine pair k/k+8; self included).
                # Data lands at the sender's absolute slot via ts(partition_id).
                # Each receiver gets +2 per sender -> +16 per step.
                #
                # Soundness of the shared counter: the remote sem alternates
                # by step parity. While a core sits at its step-t wait, no
                # peer can have issued a step-t+1 send (it would need this
                # core's step-t chunk), so only sends of steps <= t exist,
                # and step-t sends hit the OTHER parity sem. Per-engine FIFO
                # delivery makes each sender's counted prefix complete, so
                # rsem[(t-1)%2] == 16*ceil(t/2) implies every step-(t-1)
                # chunk has landed.
                #
                # Tile's single-core scheduling sim cannot see the remote
                # increments, so waits are emitted with value 0 (trivially
                # true for the scheduler), pinned in PE program order via
                # nosync deps, and the real thresholds are patched in after
                # scheduling (deferred_waits).
                rsems = [nc.alloc_semaphore(f"rdma_rsem{p}") for p in range(2)]
                lsem = nc.alloc_semaphore("rdma_lsem")
                pid_sv = nc.gpsimd.partition_id()
            deferred_waits = []
            prev_mm = None

            for t in range(t_steps):
                step_waits = []
                if exchange == "rdma" and t > 0:
                    # gate this step's matmuls on all 8 chunk arrivals
                    w = nc.tensor.wait_ge(rsems[(t - 1) % 2], 0)
                    deferred_waits.append((w, 16 * ((t + 1) // 2)))
                    step_waits.append(w)
                    if prev_mm is not None:
                        _add_dep_helper(
                            w.ins,
                            prev_mm.ins,
                            False,
                            "rdma wait after prev step",
                        )
                # one PSUM bank per gate so EW reads overlap later gates' MMs
                pst = [
                    psp.tile([128, B], F32, tag=ptags[g], name=f"ps2_{g}")
                    for g in range(4)
                ]
                # gate col order: 0=i, 1=f, 2=o, 3=g. The x_gates injects
                # don't need h_all, so they are emitted first and overlap the
                # exchange; W-groups run g first / o last so the sigmoid/mul
                # chain overlaps the remaining MM groups.
                for g in (3, 0, 1, 2):
                    nc.tensor.matmul(
                        pst[g][:],
                        id_sb[:],
                        xg[g][:, t * B : (t + 1) * B],
                        start=True,
                        stop=False,
                    )
                for gi, g in enumerate((3, 0, 1, 2)):
                    for kk in range(8):
                        mm = nc.tensor.matmul(
                            pst[g][:],
                            whh_sb[:, kk, g * 128 : (g + 1) * 128],
                            h_all[:, kk * B : (kk + 1) * B],
                            start=False,
                            stop=(kk == 7),
                        )
                        if gi == 0 and kk == 0:
                            for w in step_waits:
                                _add_dep_helper(
                                    mm.ins,
                                    w.ins,
                                    False,
                                    "first mm after rdma wait",
                                )
                    prev_mm = mm
                    if g == 3:
                        gr = ew.tile([128, B], F32, tag="gr")
                        nc.vector.tensor_scalar_max(gr[:], pst[3][:], 0.0)
                    elif g == 0:
                        sig_i = ew.tile([128, B], F32, tag="sig_i")
                        nc.scalar.activation(sig_i[:], pst[0][:], AF.Sigmoid)
                        ig = ew.tile([128, B], F32, tag="ig")
                        nc.vector.tensor_tensor(ig[:], sig_i[:], gr[:], ALU.mult)
                    elif g == 1:
                        sig_f = ew.tile([128, B], F32, tag="sig_f")
                        nc.scalar.activation(sig_f[:], pst[1][:], AF.Sigmoid)
                        fc = ew.tile([128, B], F32, tag="fc")
                        nc.vector.tensor_tensor(
                            fc[:], sig_f[:], c_prev[:], ALU.mult
                        )
                        c_new = ew.tile([128, B], F32, tag="c")
                        nc.vector.tensor_tensor(c_new[:], fc[:], ig[:], ALU.add)
                        rc = ew.tile([128, B], F32, tag="rc")
                        nc.scalar.activation(rc[:], c_new[:], AF.Relu)
                    else:
                        sig_o = ew.tile([128, B], F32, tag="sig_o")
                        nc.scalar.activation(sig_o[:], pst[2][:], AF.Sigmoid)
                hbf = ew.tile([128, B], BF, tag="hbf")
                nc.vector.tensor_tensor(hbf[:], sig_o[:], rc[:], ALU.mult)

                # exchange h^T chunks across the 8 cores
                if exchange == "rdma":
                    hn = hallp.tile([128, 8 * B], BF, tag="hall", name="hn")
                    nc.gpsimd.remote_dma_broadcast(
                        hn[:, bass.ts(pid_sv, B)],
                        hbf[:],
                        rsems[t % 2],
                        lsem,
                        rdests=[(0, k) for k in range(NCORES)],
                    )
                    nc.gpsimd.trigger_dma(count=None)
                    h_all = hn
                else:
                    ci = dram.tile([128, B], BF, tag="ccin")
                    co = dram.tile([128 * NCORES, B], BF, tag="ccout")
                    nc.sync.dma_start(ci[:], hbf[:])
                    nc.gpsimd.collective_compute(
                        "AllGather",
                        ALU.bypass,
                        replica_groups=[list(range(NCORES))],
                        ins=[ci.opt()],
                        outs=[co.opt()],
                    )
                    # split the gather readback per chunk: 8 contiguous 16KB
                    # DMAs instead of one strided 128KB one, so the first
                    # W-matmul group starts as soon as chunk 0 lands
                    h_all = hallp.tile([128, 8 * B], BF, tag="hall")
                    hv = h_all[:].rearrange("p (r b) -> p r b", r=NCORES)
                    cv = co.rearrange("(r p) b -> p r b", p=128)
                    for r in range(NCORES):
                        nc.sync.dma_start(hv[:, r, :], cv[:, r, :])
                c_prev = c_new

            # ---- phase 3: out = h @ W_o + b_o ----
            step_waits = []
            if exchange == "rdma":
                w = nc.tensor.wait_ge(rsems[(t_steps - 1) % 2], 0)
                deferred_waits.append((w, 16 * ((t_steps + 1) // 2)))
                step_waits.append(w)
                if prev_mm is not None:
                    _add_dep_helper(w.ins, prev_mm.ins, False, "ph3 wait")
            pso = psp.tile([B, O], F32, tag="pA", name="pso")
            mm = nc.tensor.matmul(
                pso[:], ones_sb[:], bo_sb[:], start=True, stop=False
            )
            for w in step_waits:
                _add_dep_helper(mm.ins, w.ins, False, "ph3 mm after wait")
            for kk in range(8):
                nc.tensor.matmul(
                    pso[:],
                    h_all[:, kk * B : (kk + 1) * B],
                    wo_sb[:, kk, :],
                    start=False,
                    stop=(kk == 7),
                )
            out_sb = ew.tile([B, O], F32, tag="osb")
            nc.vector.tensor_copy(out_sb[:], pso[:])
            nc.sync.dma_start(out[:, :], out_sb[:])

    # Patch the real remote-sem thresholds now that Tile has scheduled
    # (placement was pinned with nosync deps during emission).
    for w, val in deferred_waits:
        w.ins.sync_info.on_wait[0].wait_value = val

    nc.compile()
    return nc


def prep_inputs(xs, W_ih, W_hh, b, W_o, b_o):
    """Host-side sharding/layout. Returns in_maps for the 8 cores."""
    xs = np.asarray(xs, dtype=np.float32)
    W_ih = np.asarray(W_ih, dtype=np.float32)
    W_hh = np.asarray(W_hh, dtype=np.float32)
    b = np.asarray(b, dtype=np.float32)
    W_o = np.asarray(W_o, dtype=np.float32)
    b_o = np.asarray(b_o, dtype=np.float32)

    # xs^T in (i, t*64+b) layout, shared by all cores
    xs_t = np.ascontiguousarray(
        xs.transpose(2, 1, 0).reshape(I, T * B)
    ).astype(BF16)
    ident = np.eye(128, dtype=BF16)
    ones = np.ones((1, B), dtype=BF16)
    wo_bf = np.ascontiguousarray(W_o).astype(BF16)
    bo_bf = np.ascontiguousarray(b_o[None, :]).astype(BF16)

    in_maps = []
    for j in range(NCORES):
        # gate columns for core j, permuted to [i, f, o, g] (orig order i,f,g,o)
        cols = np.concatenate(
            [
                np.arange(g * H + j * HSL, g * H + (j + 1) * HSL)
                for g in (0, 1, 3, 2)
            ]
        )
        in_maps.append(
            {
                "xs_t": xs_t,
                "wih": np.ascontiguousarray(W_ih[:, cols]).astype(BF16),
                "whh": np.ascontiguousarray(W_hh[:, cols]).astype(BF16),
                "bias": np.ascontiguousarray(
                    b[cols].reshape(4, HSL).T
                ).astype(np.float32),
                "wo": wo_bf,
                "bo": bo_bf,
                "ident": ident,
                "ones": ones,
            }
        )
    return in_maps


_NC_CACHE = {}
EXCHANGE = "cc"


def _get_nc(t_steps: int = T, exchange: str | None = None):
    exchange = exchange or EXCHANGE
    key = (t_steps, exchange)
    if key not in _NC_CACHE:
        _NC_CACHE[key] = build_program(t_steps, exchange)
    return _NC_CACHE[key]


def _run(inputs, trace=False):
    nc = _get_nc(T)
    in_maps = prep_inputs(**inputs)
    # The fleet shows occasional transient NRT_EXEC_UNIT_UNRECOVERABLE faults
    # that clear after a short wait; retry a couple of times.
    last_err = None
    for attempt in range(3):
        try:
            res = bass_utils.run_bass_kernel_spmd(
                nc, in_maps, core_ids=list(range(NCORES)), trace=trace
            )
            out = np.asarray(res.results[0]["out"], dtype=np.float32)
            return out, res
        except Exception as e:  # noqa: BLE001 - device-transient errors
            last_err = e
            if attempt < 2:
                import time

                time.sleep(45)
    raise last_err


def kernel(**inputs) -> np.ndarray:
    out, _ = _run(inputs, trace=False)
    return out


def run_traced(**inputs):
    return _run(inputs, trace=True)



# revision 2
# speedup vs baseline: 1.1722x; 1.1722x over previous
"""Trainium2 Bass kernel for nn_LstmRNN: 8-core tensor-parallel LSTM.

Strategy (8 NeuronCores, SPMD):
  - Tensor-parallel split of the 4H gate dimension: core j owns hidden-state
    slice j (128 of 1024 dims) and the matching 512 columns of W_ih / W_hh
    (gate order permuted host-side to [i, f, o, g] so the sigmoid covers
    contiguous blocks of the transposed gates).
  - Phase 1: x_gates^T = W_ih_local^T @ xs^T for all (t, b), bf16 matmuls,
    fp32 PSUM accumulate, bias fused into the PSUM->SBUF eviction.
  - Phase 2: 128 serial steps. Per step: inject x_gates into PSUM via an
    identity matmul (emitted first; overlaps the exchange window), accumulate
    8 K-chunk matmuls of h^T @ W_hh per gate into one PSUM bank per gate,
    sigmoid (ACT) + relu/mul/add (DVE) in fp32 overlapping the later gates'
    matmul groups, produce the core's h^T chunk [128, 64] in bf16, AllGather
    it across the 8 cores (bounce DRAM -> ncfw collective -> DRAM -> SBUF).
  - Phase 3: out = h @ W_o + b_o, bias via a K=1 ones-row matmul.

All matmul operands are bf16 (1 cycle/row on PE vs 4 for fp32); state c and
all elementwise math stay fp32. Validated vs the fp32 reference: max
absmax-relative error ~2.3e-3 (gate is 2e-2).

Measured on trn2 (neuron-profile): ~2.10 ms total; per step ~15.4 us =
~2.5 us matmul+EW (PE warm, 53 ns/MM issue) + ~12.9 us exchange (the
per-step 8-core AllGather path: ~0.6+1.3 us bounce-out, ~7.2 us ncfw
collective, ~0.6+2.3 us bounce-in incl. HBM completion receipt).

An alternative exchange="rdma" path (SBUF->SBUF remote_dma_broadcast,
per-step latency ~2-3 us in theory) is implemented and validated in
MultiCoreSim; on hardware the 8x singleton-destination variant was stable
but serialized deliveries (~51 us/step), and the single 8-destination
variant faulted the device (NRT_EXEC_UNIT_UNRECOVERABLE), so the collective
exchange remains the default.
"""

import sys

for _p in ("/opt/trn_rl_repo",):
    if _p not in sys.path:
        sys.path.insert(0, _p)

import numpy as np
import ml_dtypes

import concourse.bass as bass
import concourse.mybir as mybir
import concourse.tile as tile
from concourse import bacc
from concourse import bass_utils
from concourse.bass import _add_dep_helper

BF16 = ml_dtypes.bfloat16

B, T, I, H, O = 64, 128, 512, 1024, 512
NCORES = 8
HSL = H // NCORES          # 128 hidden dims per core
GCOLS = 4 * HSL            # 512 gate columns per core (i,f,o,g x 128)
NB = T * B // 512          # phase-1 512-wide col-chunks (16)

F32 = mybir.dt.float32
BF = mybir.dt.bfloat16
AF = mybir.ActivationFunctionType
ALU = mybir.AluOpType


def build_program(t_steps: int = T, exchange: str = "rdma"):
    """exchange: 'rdma' = SBUF->SBUF remote_dma_broadcast; 'cc' = ncfw AllGather."""
    nc = bacc.Bacc(
        "TRN2",
        target_bir_lowering=False,
        debug=False,
        num_devices=NCORES,
    )

    xs_t = nc.dram_tensor("xs_t", [I, T * B], BF, kind="ExternalInput")
    wih = nc.dram_tensor("wih", [I, GCOLS], BF, kind="ExternalInput")
    whh = nc.dram_tensor("whh", [H, GCOLS], BF, kind="ExternalInput")
    bias = nc.dram_tensor("bias", [HSL, 4], F32, kind="ExternalInput")
    wo = nc.dram_tensor("wo", [H, O], BF, kind="ExternalInput")
    bo = nc.dram_tensor("bo", [1, O], BF, kind="ExternalInput")
    ident = nc.dram_tensor("ident", [128, 128], BF, kind="ExternalInput")
    ones = nc.dram_tensor("ones", [1, B], BF, kind="ExternalInput")
    out = nc.dram_tensor("out", [B, O], F32, kind="ExternalOutput")

    with tile.TileContext(nc) as tc:
        with (
            tc.tile_pool(name="consts", bufs=1) as consts,
            tc.tile_pool(name="xg", bufs=1) as xgp,
            tc.tile_pool(name="xsp", bufs=2) as xsp,
            tc.tile_pool(name="psum", bufs=2, space="PSUM") as psp,
            tc.tile_pool(name="ew", bufs=2) as ew,
            tc.tile_pool(name="hall", bufs=2) as hallp,
            tc.tile_pool(name="dram", bufs=2, space="DRAM") as dram,
        ):
            # ---- constants into SBUF ----
            wih_sb = consts.tile([128, 4, GCOLS], BF)
            nc.sync.dma_start(wih_sb[:], wih.rearrange("(k p) c -> p k c", p=128))
            whh_sb = consts.tile([128, 8, GCOLS], BF)
            nc.sync.dma_start(whh_sb[:], whh.rearrange("(k p) c -> p k c", p=128))
            bias_sb = consts.tile([HSL, 4], F32)
            nc.sync.dma_start(bias_sb[:], bias[:, :])
            id_sb = consts.tile([128, 128], BF)
            nc.sync.dma_start(id_sb[:], ident[:, :])
            ones_sb = consts.tile([1, B], BF)
            nc.sync.dma_start(ones_sb[:], ones[:, :])
            wo_sb = consts.tile([128, 8, O], BF)
            nc.sync.dma_start(wo_sb[:], wo.rearrange("(k p) c -> p k c", p=128))
            bo_sb = consts.tile([1, O], BF)
            nc.sync.dma_start(bo_sb[:], bo[:, :])

            # x_gates^T per gate chunk: [128 gate-dims, t*64+b]
            xg = [
                xgp.tile([128, T * B], BF, tag=f"xg{g}", name=f"xg{g}")
                for g in range(4)
            ]

            # ---- phase 1: x_gates^T = W_ih_local^T @ xs^T (+ bias) ----
            # PSUM budget: 4 tags x bufs=2 = 8 banks, shared with phase 2/3.
            ptags = ["pA", "pB", "pC", "pD"]
            xs_r = xs_t.rearrange("(k p) n -> p k n", p=128)
            for blk in range(T * B // 1024):  # 8 panels of 1024 cols
                panel = xsp.tile([128, 4, 1024], BF, tag="panel")
                nc.sync.dma_start(
                    panel[:], xs_r[:, :, blk * 1024 : (blk + 1) * 1024]
                )
                for g in range(4):
                    for sub in range(2):
                        ps = psp.tile(
                            [128, 512], F32, tag=ptags[g], name=f"ps1_{g}"
                        )
                        for k in range(4):
                            nc.tensor.matmul(
                                ps[:],
                                wih_sb[:, k, g * 128 : (g + 1) * 128],
                                panel[:, k, sub * 512 : (sub + 1) * 512],
                                start=(k == 0),
                                stop=(k == 3),
                            )
                        col0 = blk * 1024 + sub * 512
                        nc.vector.tensor_scalar(
                            xg[g][:, col0 : col0 + 512],
                            ps[:],
                            bias_sb[:, g : g + 1],
                            None,
                            ALU.add,
                        )

            # ---- phase 2: recurrence ----
            h_all = hallp.tile([128, 8 * B], BF, tag="hall")
            nc.vector.memset(h_all[:], 0.0)
            c_prev = ew.tile([128, B], F32, tag="c")
            nc.vector.memset(c_prev[:], 0.0)

            if exchange == "rdma":
                # remote-DMA all-gather: per step, ONE 8-destination broadcast
                # (relative peers own^k on engine pair k/k+8; self included).
                # Data lands at the sender's absolute slot via ts(partition_id).
                # Each receiver gets +2 per sender -> +16 per step.
                #
                # Soundness of the shared counter: the remote sem alternates
                # by step parity. While a core sits at its step-t wait, no
                # peer can have issued a step-t+1 send (it would need this
                # core's step-t chunk), so only sends of steps <= t exist,
                # and step-t sends hit the OTHER parity sem. Per-engine FIFO
                # delivery makes each sender's counted prefix complete, so
                # rsem[(t-1)%2] == 16*ceil(t/2) implies every step-(t-1)
                # chunk has landed.
                #
                # Tile's single-core scheduling sim cannot see the remote
                # increments, so waits are emitted with value 0 (trivially
                # true for the scheduler), pinned in PE program order via
                # nosync deps, and the real thresholds are patched in after
                # scheduling (deferred_waits).
                rsems = [nc.alloc_semaphore(f"rdma_rsem{p}") for p in range(2)]
                lsem = nc.alloc_semaphore("rdma_lsem")
                pid_sv = nc.gpsimd.partition_id()
            deferred_waits = []
            prev_mm = None

            for t in range(t_steps):
                step_waits = []
                if exchange == "rdma" and t > 0:
                    # gate this step's matmuls on all 8 chunk arrivals
                    w = nc.tensor.wait_ge(rsems[(t - 1) % 2], 0)
                    deferred_waits.append((w, 16 * ((t + 1) // 2)))
                    step_waits.append(w)
                    if prev_mm is not None:
                        _add_dep_helper(
                            w.ins,
                            prev_mm.ins,
                            False,
                            "rdma wait after prev step",
                        )
                # one PSUM bank per gate so EW reads overlap later gates' MMs
                pst = [
                    psp.tile([128, B], F32, tag=ptags[g], name=f"ps2_{g}")
                    for g in range(4)
                ]
                # gate col order: 0=i, 1=f, 2=o, 3=g. The x_gates injects
                # don't need h_all, so they are emitted first and overlap the
                # exchange; W-groups run g first / o last so the sigmoid/mul
                # chain overlaps the remaining MM groups.
                for g in (3, 0, 1, 2):
                    nc.tensor.matmul(
                        pst[g][:],
                        id_sb[:],
                        xg[g][:, t * B : (t + 1) * B],
                        start=True,
                        stop=False,
                    )
                for gi, g in enumerate((3, 0, 1, 2)):
                    for kk in range(8):
                        mm = nc.tensor.matmul(
                            pst[g][:],
                            whh_sb[:, kk, g * 128 : (g + 1) * 128],
                            h_all[:, kk * B : (kk + 1) * B],
                            start=False,
                            stop=(kk == 7),
                        )
                        if gi == 0 and kk == 0:
                            for w in step_waits:
                                _add_dep_helper(
                                    mm.ins,
                                    w.ins,
                                    False,
                                    "first mm after rdma wait",
                                )
                    prev_mm = mm
                    if g == 3:
                        gr = ew.tile([128, B], F32, tag="gr")
                        nc.vector.tensor_scalar_max(gr[:], pst[3][:], 0.0)
                    elif g == 0:
                        sig_i = ew.tile([128, B], F32, tag="sig_i")
                        nc.scalar.activation(sig_i[:], pst[0][:], AF.Sigmoid)
                        ig = ew.tile([128, B], F32, tag="ig")
                        nc.vector.tensor_tensor(ig[:], sig_i[:], gr[:], ALU.mult)
                    elif g == 1:
                        sig_f = ew.tile([128, B], F32, tag="sig_f")
                        nc.scalar.activation(sig_f[:], pst[1][:], AF.Sigmoid)
                        fc = ew.tile([128, B], F32, tag="fc")
                        nc.vector.tensor_tensor(
                            fc[:], sig_f[:], c_prev[:], ALU.mult
                        )
                        c_new = ew.tile([128, B], F32, tag="c")
                        nc.vector.tensor_tensor(c_new[:], fc[:], ig[:], ALU.add)
                        rc = ew.tile([128, B], F32, tag="rc")
                        nc.scalar.activation(rc[:], c_new[:], AF.Relu)
                    else:
                        sig_o = ew.tile([128, B], F32, tag="sig_o")
                        nc.scalar.activation(sig_o[:], pst[2][:], AF.Sigmoid)
                hbf = ew.tile([128, B], BF, tag="hbf")
                nc.vector.tensor_tensor(hbf[:], sig_o[:], rc[:], ALU.mult)

                # exchange h^T chunks across the 8 cores
                if exchange == "rdma":
                    hn = hallp.tile([128, 8 * B], BF, tag="hall", name="hn")
                    nc.gpsimd.remote_dma_broadcast(
                        hn[:, bass.ts(pid_sv, B)],
                        hbf[:],
                        rsems[t % 2],
                        lsem,
                        rdests=[(0, k) for k in range(NCORES)],
                    )
                    nc.gpsimd.trigger_dma(count=None)
                    h_all = hn
                else:
                    ci = dram.tile([128, B], BF, tag="ccin")
                    co = dram.tile([128 * NCORES, B], BF, tag="ccout")
                    nc.sync.dma_start(ci[:], hbf[:])
                    nc.gpsimd.collective_compute(
                        "AllGather",
                        ALU.bypass,
                        replica_groups=[list(range(NCORES))],
                        ins=[ci.opt()],
                        outs=[co.opt()],
                    )
                    # one strided 128KB readback: the 8 per-chunk DMAs
                    # serialize on the Sync HWDGE queue (~650ns apart = 5.2us
                    # and they pace the W-matmul groups); a single DMA lands
                    # everything in ~1.2us and lets the gate-outer matmul
                    # order overlap the EW chain
                    h_all = hallp.tile([128, 8 * B], BF, tag="hall")
                    hv = h_all[:].rearrange("p (r b) -> p r b", r=NCORES)
                    cv = co.rearrange("(r p) b -> p r b", p=128)
                    nc.sync.dma_start(hv[:, :, :], cv[:, :, :])
                c_prev = c_new

            # ---- phase 3: out = h @ W_o + b_o ----
            step_waits = []
            if exchange == "rdma":
                w = nc.tensor.wait_ge(rsems[(t_steps - 1) % 2], 0)
                deferred_waits.append((w, 16 * ((t_steps + 1) // 2)))
                step_waits.append(w)
                if prev_mm is not None:
                    _add_dep_helper(w.ins, prev_mm.ins, False, "ph3 wait")
            pso = psp.tile([B, O], F32, tag="pA", name="pso")
            mm = nc.tensor.matmul(
                pso[:], ones_sb[:], bo_sb[:], start=True, stop=False
            )
            for w in step_waits:
                _add_dep_helper(mm.ins, w.ins, False, "ph3 mm after wait")
            for kk in range(8):
                nc.tensor.matmul(
                    pso[:],
                    h_all[:, kk * B : (kk + 1) * B],
                    wo_sb[:, kk, :],
                    start=False,
                    stop=(kk == 7),
                )
            out_sb = ew.tile([B, O], F32, tag="osb")
            nc.vector.tensor_copy(out_sb[:], pso[:])
            nc.sync.dma_start(out[:, :], out_sb[:])

    # Patch the real remote-sem thresholds now that Tile has scheduled
    # (placement was pinned with nosync deps during emission).
    for w, val in deferred_waits:
        w.ins.sync_info.on_wait[0].wait_value = val

    nc.compile()
    return nc


def prep_inputs(xs, W_ih, W_hh, b, W_o, b_o):
    """Host-side sharding/layout. Returns in_maps for the 8 cores."""
    xs = np.asarray(xs, dtype=np.float32)
    W_ih = np.asarray(W_ih, dtype=np.float32)
    W_hh = np.asarray(W_hh, dtype=np.float32)
    b = np.asarray(b, dtype=np.float32)
    W_o = np.asarray(W_o, dtype=np.float32)
    b_o = np.asarray(b_o, dtype=np.float32)

    # xs^T in (i, t*64+b) layout, shared by all cores
    xs_t = np.ascontiguousarray(
        xs.transpose(2, 1, 0).reshape(I, T * B)
    ).astype(BF16)
    ident = np.eye(128, dtype=BF16)
    ones = np.ones((1, B), dtype=BF16)
    wo_bf = np.ascontiguousarray(W_o).astype(BF16)
    bo_bf = np.ascontiguousarray(b_o[None, :]).astype(BF16)

    in_maps = []
    for j in range(NCORES):
        # gate columns for core j, permuted to [i, f, o, g] (orig order i,f,g,o)
        cols = np.concatenate(
            [
                np.arange(g * H + j * HSL, g * H + (j + 1) * HSL)
                for g in (0, 1, 3, 2)
            ]
        )
        in_maps.append(
            {
                "xs_t": xs_t,
                "wih": np.ascontiguousarray(W_ih[:, cols]).astype(BF16),
                "whh": np.ascontiguousarray(W_hh[:, cols]).astype(BF16),
                "bias": np.ascontiguousarray(
                    b[cols].reshape(4, HSL).T
                ).astype(np.float32),
                "wo": wo_bf,
                "bo": bo_bf,
                "ident": ident,
                "ones": ones,
            }
        )
    return in_maps


_NC_CACHE = {}
EXCHANGE = "cc"


def _get_nc(t_steps: int = T, exchange: str | None = None):
    exchange = exchange or EXCHANGE
    key = (t_steps, exchange)
    if key not in _NC_CACHE:
        _NC_CACHE[key] = build_program(t_steps, exchange)
    return _NC_CACHE[key]


def _run(inputs, trace=False):
    nc = _get_nc(T)
    in_maps = prep_inputs(**inputs)
    # The fleet shows occasional transient NRT_EXEC_UNIT_UNRECOVERABLE faults
    # that clear after a short wait; retry a couple of times.
    last_err = None
    for attempt in range(3):
        try:
            res = bass_utils.run_bass_kernel_spmd(
                nc, in_maps, core_ids=list(range(NCORES)), trace=trace
            )
            out = np.asarray(res.results[0]["out"], dtype=np.float32)
            return out, res
        except Exception as e:  # noqa: BLE001 - device-transient errors
            last_err = e
            if attempt < 2:
                import time

                time.sleep(45)
    raise last_err


def kernel(**inputs) -> np.ndarray:
    out, _ = _run(inputs, trace=False)
    return out


def run_traced(**inputs):
    return _run(inputs, trace=True)

